# revision 75
# baseline (speedup 1.0000x reference)
"""Trainium2 Bass kernel for an AttNHP transformer layer.

Shapes (hardcoded): src (4, 1024, 512), nhead=8 with full-width (512) q/k per
head, dim_value 64, ffn 2048.  Runs SPMD on 8 NeuronCores: core c handles
batch c//2, query-token half c%2.  The host rotates the token axis per core so
one shared program works for every core, computes the combined additive
attention-mask bias, and pre-transposes all weights.

Attention trick: scores[l,m] = q_l.k_m with q=Wq s+bq, k=Wk s+bk equals
s_l^T A s_m + g(m) + f(l) + const with A = Wq^T Wk and g(m) = (Wk^T bq).s_m.
The f(l)+const part is constant along the softmax axis and cancels exactly,
so the device only computes u = A^T s (own tokens) and scores = u.s — the
whole K projection disappears.  A and c = Wk^T bq are precomputed on host.
All heavy matmuls run in bf16 (fast-weight-load eligible); residual/LN math
stays fp32.
"""

import os
import sys
import numpy as np
from contextlib import ExitStack

for _p in ("/opt/trn_rl_repo", "/root/.axon_site/_ro/trn_rl_repo"):
    if os.path.isdir(_p) and _p not in sys.path:
        sys.path.append(_p)

import concourse.bass as bass  # noqa: E402
import concourse.tile as tile  # noqa: E402
from concourse import bacc, mybir  # noqa: E402
from concourse.bass_utils import run_bass_kernel_spmd  # noqa: E402

import ml_dtypes  # noqa: E402

f32 = mybir.dt.float32
f32r = mybir.dt.float32r
bf16 = mybir.dt.bfloat16
f8 = mybir.dt.float8e4
AF = mybir.ActivationFunctionType
ALU = mybir.AluOpType
DR = mybir.MatmulPerfMode.DoubleRow

B, L, D, H, DV, F = 4, 1024, 512, 8, 64, 2048
LQ = L // 2          # tokens per core
NCORES = 8
NDT = D // 128       # 4  d-tiles
NMT = L // 128       # 8  key-token tiles
NFT = F // 128       # 16 ffn tiles
SCALE = 1.0 / float(np.sqrt(np.float32(D)))
NINF = -1000000.0
BF = ml_dtypes.bfloat16
F8 = ml_dtypes.float8_e4m3
ASCALE = 64.0   # A,c,biasT scaled by this; compensated in the exp scale
WSCALE = 64.0   # ff1 weight scale into fp8 normal range
XSCALE = 16.0   # LN1 output fold (x2/LN2 are scale-invariant)

_PROG_CACHE = {}
_ACT_PATCHED = [False]


def _patch_act_tables():
    """Force every ACTIVATE onto the natural_log_exp_and_others table set.

    The default chooser maps each function to its first-containing set
    (Exp -> exp_and_others, Ln -> natural_log), which reloads tables twice
    per attention head.  Emptying every other set (order, hence set ids,
    preserved) makes the one set that contains Exp+Ln+Relu+Identity+Square
    the only candidate: one ACT_TABLE_LOAD for the whole kernel.
    """
    if _ACT_PATCHED[0]:
        return
    orig = bacc.get_activation_tables

    def patched(arch):
        t = dict(orig(arch))
        keep = t.get("natural_log_exp_and_others")
        if not keep:
            return t
        for k in t:
            if k != "natural_log_exp_and_others":
                t[k] = t[k] - keep
        return t

    bacc.get_activation_tables = patched
    _ACT_PATCHED[0] = True


def _build_program():
    key = "prog"
    if key in _PROG_CACHE:
        return _PROG_CACHE[key]
    _patch_act_tables()

    nc = bacc.Bacc("TRN2", target_bir_lowering=False, debug=False,
                   num_devices=NCORES)

    def din(name, shape, dt=f32):
        return nc.dram_tensor(name, list(shape), dt, kind="ExternalInput").ap()

    srcT_d = din("srcT", (D, L))
    src8_d = din("src8", (128, 4 * L), f8)   # [ki, (dp*2+ko)*L + l] packed
    srcb_d = din("srcb", (D, L), bf16)
    biasT_d = din("biasT", (L, LQ), bf16)
    aT_d = din("aT", (D, H * D), f8)         # ASCALE * Wq_h^T Wk_h
    cT_d = din("cT", (128, NDT * H), bf16)   # ASCALE * Wk_h^T bq_h, tiled
    wvT_d = din("wvT", (D, H * DV), bf16)
    bvbc_d = din("bvbc", (128, H * DV))
    ff1T_d = din("ff1T", (D, F), f8)         # WSCALE * ff1_w*ln1_g, transposed
    ff2T_d = din("ff2T", (F, D), f8)         # XSCALE * ff2_w, transposed
    ff1b_d = din("ff1b", (128, NFT))
    ff2b_d = din("ff2b", (128, NDT))
    ln2g_d = din("ln2g", (128, NDT))
    ln2b_d = din("ln2b", (128, NDT))
    outT_d = nc.dram_tensor("outT", [D, LQ], f32, kind="ExternalOutput").ap()

    with tile.TileContext(nc) as tc, ExitStack() as ctx:
        pp = ctx.enter_context(tc.tile_pool(name="pp", bufs=1))
        ps = ctx.enter_context(tc.tile_pool(name="ps", bufs=6, space="PSUM"))
        ps_st = ctx.enter_context(tc.tile_pool(name="psst", bufs=1,
                                               space="PSUM"))
        ap_x1 = ctx.enter_context(tc.tile_pool(name="x1p", bufs=1))
        ap_sq = ctx.enter_context(tc.tile_pool(name="sqp", bufs=2))

        def load_const(name, dram, shape, dt=f32):
            t = pp.tile(list(shape), dt, name=name, tag=name)
            nc.sync.dma_start(t[:], dram[:])
            return t

        onesf = pp.tile([128, 8], bf16, name="onesf", tag="onesf")
        nc.vector.memset(onesf[:], 1.0)
        onesc = pp.tile([128, 1], f32, name="onesc", tag="onesc")
        nc.vector.memset(onesc[:], 1.0)
        ones_col = pp.tile([128, 1], f32r, name="ones", tag="ones")
        nc.vector.tensor_copy(ones_col[:], onesc[:])
        onesr = pp.tile([1, 128], f32, name="onesr", tag="onesr")
        nc.vector.memset(onesr[:], 1.0)
        ones_row = pp.tile([1, 128], f32r, name="onesrr", tag="onesrr")
        nc.vector.tensor_copy(ones_row[:], onesr[:])
        eps30 = pp.tile([1, 1], f32, name="eps30", tag="eps30")
        nc.vector.memset(eps30[:], 1e-30)
        epsln = pp.tile([1, 1], f32, name="epsln", tag="epsln")
        nc.vector.memset(epsln[:], 1e-5)
        lnxs = pp.tile([1, 1], f32, name="lnxs", tag="lnxs")
        nc.vector.memset(lnxs[:], float(np.log(XSCALE)))
        lnws = pp.tile([1, 1], f32, name="lnws", tag="lnws")
        nc.vector.memset(lnws[:], float(np.log(WSCALE * XSCALE)))

        # src arrives pre-cast from the host: fp8 DoubleRow-packed (uproj
        # moving + scores stationary; src8[dp][ki,ko,l] = src[dp*256 +
        # ko*128 + ki, l]), bf16 (vproj/g), and fp32 (residual path, loaded
        # late since it is first read at flush_norm of head 1).
        src8 = [pp.tile([128, 2, L], f8, name=f"src8{dp}", tag=f"src8{dp}")
                for dp in range(2)]
        for dp in range(2):
            for ko in range(2):
                c0 = (dp * 2 + ko) * L
                nc.sync.dma_start(src8[dp][:, ko, 0:LQ],
                                  src8_d[:, c0:c0 + LQ])
        srcT = []
        srcb = []
        for dt in range(NDT):
            t = pp.tile([128, LQ], f32, name=f"srcT{dt}", tag=f"srcT{dt}")
            srcT.append(t)
            b = pp.tile([128, L], bf16, name=f"srcb{dt}", tag=f"srcb{dt}")
            nc.sync.dma_start(b[:, 0:LQ], srcb_d[dt * 128:(dt + 1) * 128, 0:LQ])
            srcb.append(b)

        vaug = [pp.tile([128, H * 65], bf16, name=f"vaug{mt}", tag=f"vaug{mt}")
                for mt in range(NMT)]
        saT = [pp.tile([128, LQ], f32, name=f"saT{dt}", tag=f"saT{dt}")
               for dt in range(NDT)]
        biasT = [pp.tile([128, LQ], bf16, name=f"biasT{mt}", tag=f"biasT{mt}")
                 for mt in range(NMT)]
        g_sb = pp.tile([128, NMT * H], f32, name="g_sb", tag="g_sb")

        with ExitStack() as actx:
            ap_wa = actx.enter_context(tc.tile_pool(name="wap", bufs=8))
            ap_u = actx.enter_context(tc.tile_pool(name="up", bufs=8))
            ap_ex = actx.enter_context(tc.tile_pool(name="exq", bufs=8))
            ap_sc = actx.enter_context(tc.tile_pool(name="scq", bufs=4))
            ap_row = actx.enter_context(tc.tile_pool(name="rowp", bufs=2))

            def emit_uproj(h, mid_emit=None):
                a_t = []
                for dp in range(2):
                    t = ap_wa.tile([128, 2, D], f8, name="wa", tag="wa")
                    for ko in range(2):
                        r0 = dp * 256 + ko * 128
                        nc.sync.dma_start(t[:, ko, :],
                                          aT_d[r0:r0 + 128, h * D:(h + 1) * D])
                    a_t.append(t)
                u = [ap_u.tile([128, 2, LQ], f8, name="uh", tag="uh")
                     for _ in range(2)]
                for pt in range(NDT):
                    pq = ps.tile([128, LQ], f32, name="psw", tag="ps")
                    for dp in range(2):
                        nc.tensor.matmul(pq[:],
                                         a_t[dp][:, :, pt * 128:(pt + 1) * 128],
                                         src8[dp][:, :, 0:LQ],
                                         start=(dp == 0), stop=(dp == 1),
                                         perf_mode=DR)
                    if pt % 2 == 0:
                        nc.scalar.activation(u[pt // 2][:, pt % 2, :], pq[:],
                                             AF.Identity)
                    else:
                        nc.vector.tensor_copy(u[pt // 2][:, pt % 2, :], pq[:])
                    if mid_emit is not None and pt == 0:
                        mid_emit()
                return u

            def emit_scores(h, u):
                ex = []
                for mt in range(NMT):
                    psc = ps.tile([128, LQ], f32, name="psw", tag="ps")
                    for dp in range(2):
                        nc.tensor.matmul(psc[:],
                                         src8[dp][:, :, mt * 128:(mt + 1) * 128],
                                         u[dp][:, :, :],
                                         start=(dp == 0), stop=(dp == 1),
                                         perf_mode=DR)
                    sct = ap_sc.tile([128, LQ], bf16, name="sc", tag="sc")
                    nc.vector.scalar_tensor_tensor(
                        sct[:], psc[:], g_sb[:, mt * H + h:mt * H + h + 1],
                        biasT[mt][:], ALU.add, ALU.add)
                    et = ap_ex.tile([128, LQ], bf16, name="ex", tag="ex")
                    nc.scalar.activation(et[:], sct[:], AF.Exp,
                                         scale=SCALE / ASCALE)
                    ex.append(et)
                return ex

            def dummy_mm(moving, f32_=False):
                """Tiny dead matmul: keeps the PE HAM activity window busy
                through serial DVE/ACT chains so the clock is not
                re-throttled to 1.2 GHz right before the next matmul burst.
                `moving` must be a [1, >=64] (row) or [128, >=64] AP."""
                p = moving.partition_size()
                dmt = ps.tile([1, 64], f32, name="dum", tag="ps")
                if f32_:
                    stat = eps30[:] if p == 1 else onesc[0:p, :]
                else:
                    stat = ones_row[0:1, 0:1] if p == 1 else ones_col[0:p, :]
                nc.tensor.matmul(dmt[:], stat, moving[:, 0:64],
                                 start=True, stop=True)

            def emit_pv_mm(h, ex, warm=False):
                ppv = ps.tile([65, LQ], f32, name="ppv", tag="ps")
                for mt in range(NMT):
                    nc.tensor.matmul(ppv[:], vaug[mt][:, h * 65:(h + 1) * 65],
                                     ex[mt][:], start=(mt == 0), stop=(mt == NMT - 1))
                lt = ap_row.tile([1, LQ], f32, name="lt", tag="lt")
                nc.scalar.activation(lt[:], ppv[64:65, :], AF.Ln, bias=eps30[:])
                if warm:
                    dummy_mm(lt[0:1, :], f32_=True)
                rt = ap_row.tile([1, LQ], f32r, name="rt", tag="rt")
                nc.scalar.activation(rt[:], lt[:], AF.Exp, scale=-1.0)
                return ppv, rt

            def emit_norm(h, ppv, rt, warm=False):
                prb = ps.tile([64, LQ], f32, name="prb", tag="ps")
                nc.tensor.matmul(prb[:], ones_row[0:1, 0:64], rt[:],
                                 start=True, stop=True)
                rbc = ap_row.tile([64, LQ], f32, name="rbc", tag="rbc")
                nc.vector.tensor_copy(rbc[:], prb[:])
                if warm:
                    dummy_mm(rbc[:], f32_=True)
                sat = saT[h // 2]
                r0 = (h % 2) * 64
                nc.vector.tensor_tensor(sat[r0:r0 + 64, :], ppv[0:64, :], rbc[:],
                                        ALU.mult)

            # ---- startup-ordered emission ----
            def _late_src():
                for dp in range(2):
                    for ko in range(2):
                        c0 = (dp * 2 + ko) * L
                        nc.sync.dma_start(src8[dp][:, ko, LQ:L],
                                          src8_d[:, c0 + LQ:c0 + L])
                for dt in range(NDT):
                    nc.sync.dma_start(srcb[dt][:, LQ:L],
                                      srcb_d[dt * 128:(dt + 1) * 128, LQ:L])

            u0 = emit_uproj(0, mid_emit=_late_src)

            # g(m): per-head additive score bias from Wk^T bq
            cT_sb = load_const("cT", cT_d, (128, NDT * H), bf16)
            g_ps = ps.tile([128, NMT * H], f32, name="gps", tag="ps")
            for mt in range(NMT):
                for dt in range(NDT):
                    nc.tensor.matmul(g_ps[:, mt * H:(mt + 1) * H],
                                     srcb[dt][:, mt * 128:(mt + 1) * 128],
                                     cT_sb[:, dt * H:(dt + 1) * H],
                                     start=(dt == 0), stop=(dt == NDT - 1))
            nc.vector.tensor_copy(g_sb[:], g_ps[:])

            # V projection (natural layout [m, j]) + ones column
            wv = []
            for dt in range(NDT):
                t = pp.tile([128, H * DV], bf16, name=f"wv{dt}", tag=f"wv{dt}")
                nc.sync.dma_start(t[:], wvT_d[dt * 128:(dt + 1) * 128, :])
                wv.append(t)
            bvbc_sb = load_const("bvbc", bvbc_d, (128, H * DV))
            for mt in range(NMT):
                pv = ps.tile([128, H * DV], f32, name="psv", tag="ps")
                for dt in range(NDT):
                    nc.tensor.matmul(pv[:], srcb[dt][:, mt * 128:(mt + 1) * 128],
                                     wv[dt][:], start=(dt == 0), stop=(dt == NDT - 1))
                va_v = vaug[mt][:].rearrange("p (h c) -> p h c", c=65)[:, :, 0:64]
                pv_v = pv[:].rearrange("p (h c) -> p h c", c=64)
                bv_v = bvbc_sb[:].rearrange("p (h c) -> p h c", c=64)
                nc.vector.tensor_tensor(va_v, pv_v, bv_v, ALU.add)
                va_ones = vaug[mt][:].rearrange("p (h c) -> p h c", c=65)[:, :, 64:65]
                nc.vector.tensor_copy(va_ones,
                                      onesf[:].rearrange("p (h o) -> p h o", o=1))

            # remaining big/late loads
            for mt in range(NMT):
                nc.sync.dma_start(biasT[mt][:],
                                  biasT_d[mt * 128:(mt + 1) * 128, :])
            # f32 residual src: first read at flush_norm of head 1
            for dt in range(NDT):
                nc.sync.dma_start(srcT[dt][:],
                                  srcT_d[dt * 128:(dt + 1) * 128, 0:LQ])
            ff1b_sb = load_const("ff1b", ff1b_d, (128, NFT))
            ff2b_sb = load_const("ff2b", ff2b_d, (128, NDT))
            ln2g_sb = load_const("ln2g", ln2g_d, (128, NDT))
            ln2b_sb = load_const("ln2b", ln2b_d, (128, NDT))
            # all FFN weights resident in SBUF (fp8, DoubleRow-packed); DMA
            # issue is chunked and interleaved into the head loop so per-head
            # aT prefetches are not stuck behind the weight traffic.
            ff1_sb = [pp.tile([128, 2, F], f8, name=f"ff1w{dp}", tag=f"ff1w{dp}")
                      for dp in range(2)]
            ff2_sb = [pp.tile([128, 2, D], f8, name=f"ff2w{fp_}", tag=f"ff2w{fp_}")
                      for fp_ in range(NFT // 2)]
            ff_chunks = []
            for dp in range(2):
                for ko in range(2):
                    r0 = dp * 256 + ko * 128
                    for hf in range(2):
                        ff_chunks.append(
                            (ff1_sb[dp][:, ko, hf * 1024:(hf + 1) * 1024],
                             ff1T_d[r0:r0 + 128, hf * 1024:(hf + 1) * 1024]))
            for fp_ in range(NFT // 2):
                for ko in range(2):
                    r0 = fp_ * 256 + ko * 128
                    ff_chunks.append((ff2_sb[fp_][:, ko, :],
                                      ff2T_d[r0:r0 + 128, :]))

            def issue_ff_chunks(n):
                while n > 0 and ff_chunks:
                    dst, src_ = ff_chunks.pop(0)
                    nc.sync.dma_start(dst, src_)
                    n -= 1

            # heads pipelined: scores(h) -> norm(h-1) -> uproj(h+1) -> pv(h)
            # LN1 stats matmuls stream into the head loop as x1 tiles finish.
            psx = ps_st.tile([1, LQ], f32, name="st1x", tag="stx")
            pss = ps_st.tile([1, LQ], f32, name="st1s", tag="sts")
            x1 = [None] * NDT
            sqs = [None] * NDT
            u = u0
            pend = None

            def flush_norm(warm=False):
                hp_, ppv_, rt_ = pend
                emit_norm(hp_, ppv_, rt_, warm=warm)
                if hp_ % 2 == 1:
                    dt = hp_ // 2
                    t = ap_x1.tile([128, LQ], f32r, name=f"x1{dt}", tag=f"x1{dt}")
                    nc.vector.tensor_tensor(t[:], srcT[dt][:],
                                            saT[dt][:], ALU.add)
                    x1[dt] = t
                    if warm:
                        dummy_mm(t[:])
                    sqt = ap_sq.tile([128, LQ], f32r, name=f"sq{dt}",
                                     tag=f"sq{dt}", bufs=1)
                    nc.scalar.activation(sqt[:], t[:].bitcast(f32), AF.Square)
                    sqs[dt] = sqt
                    nc.tensor.matmul(psx, ones_col[:], t[:],
                                     start=(dt == 0), stop=(dt == NDT - 1))
                    nc.tensor.matmul(pss, ones_col[:], sqt[:],
                                     start=(dt == 0), stop=(dt == NDT - 1))

            for h in range(H):
                ex = emit_scores(h, u)
                if pend is not None:
                    flush_norm()
                u = emit_uproj(h + 1) if h + 1 < H else None
                issue_ff_chunks(4)
                ppv, rt = emit_pv_mm(h, ex, warm=(h == H - 1))
                pend = (h, ppv, rt)
            flush_norm(warm=True)
            issue_ff_chunks(len(ff_chunks))

        # LN1 tail (stats -> mean/rstd -> apply), then FFN+LN2 in two
        # 256-token halves so half 0's serial tail overlaps half 1's matmuls.
        if True:
            with ExitStack() as fctx:
                fpp = fctx.enter_context(tc.tile_pool(name="fpp", bufs=1))
                fp = fctx.enter_context(tc.tile_pool(name="fp", bufs=2))
                hp = fctx.enter_context(tc.tile_pool(name="hp", bufs=4))
                LH = LQ // 2

                def ln_tail(psx_, pss_, xs, ln_, g_sb_, b_sb_, out_aps,
                            rstd_bias=None, spec=None):
                    comb = fp.tile([1, 2 * ln_], f32r, name="comb", tag="comb")
                    mean = comb[0:1, 0:ln_]
                    nc.vector.tensor_scalar(mean, psx_, 1.0 / D, None,
                                            ALU.mult)
                    pmb = ps.tile([128, ln_], f32, name="pmb", tag="ps")
                    nc.tensor.matmul(pmb[:], ones_row[:], comb[0:1, 0:ln_],
                                     start=True, stop=True)
                    m2 = fp.tile([1, ln_], f32, name="m2", tag="m2")
                    nc.vector.tensor_tensor(m2[:], mean.bitcast(f32),
                                            mean.bitcast(f32), ALU.mult)
                    var = fp.tile([1, ln_], f32, name="var", tag="var")
                    nc.vector.scalar_tensor_tensor(var[:], pss_, 1.0 / D,
                                                   m2[:], ALU.mult, ALU.subtract)
                    lnv = fp.tile([1, ln_], f32, name="lnv", tag="lnv")
                    nc.scalar.activation(lnv[:], var[:], AF.Ln, bias=epsln[:])
                    if spec is not None:
                        # R rows for the speculative-ff1 rank-1 fixup:
                        # row0 = -XSCALE*mean, row1 = (WSCALE*XSCALE)/r.
                        # Row 1 must land on partition 1 — engines can only
                        # write partition-0-based APs, so hop via DMA.
                        nc.vector.tensor_scalar(spec[0:1, :],
                                                mean.bitcast(f32), -XSCALE,
                                                None, ALU.mult)
                        r1t = fp.tile([1, ln_], bf16, name="r1t", tag="r1t")
                        nc.scalar.activation(r1t[:], lnv[:], AF.Exp,
                                             scale=0.5, bias=lnws[:])
                        nc.sync.dma_start(spec[1:2, :], r1t[:])
                    nc.scalar.activation(comb[0:1, ln_:2 * ln_], lnv[:], AF.Exp,
                                         scale=-0.5,
                                         bias=0.0 if rstd_bias is None
                                         else rstd_bias)
                    prs = ps.tile([128, ln_], f32, name="prs", tag="ps")
                    nc.tensor.matmul(prs[:], ones_row[:], comb[0:1, ln_:2 * ln_],
                                     start=True, stop=True)
                    for dt in range(NDT):
                        t1 = fp.tile([128, ln_], f32, name="lnt1", tag="lnt1")
                        nc.vector.tensor_tensor(t1[:], xs[dt].bitcast(f32),
                                                pmb[:], ALU.subtract)
                        if g_sb_ is None:
                            nc.vector.tensor_tensor(out_aps[dt], t1[:], prs[:],
                                                    ALU.mult)
                        else:
                            t2 = fp.tile([128, ln_], f32, name="lnt2", tag="lnt2")
                            nc.vector.tensor_tensor(t2[:], t1[:], prs[:], ALU.mult)
                            nc.scalar.activation(out_aps[dt], t2[:], AF.Identity,
                                                 bias=b_sb_[:, dt:dt + 1],
                                                 scale=g_sb_[:, dt:dt + 1])
                    return prs

                xTb = [fpp.tile([128, LQ], bf16, name=f"xTb{dt}", tag=f"xTb{dt}")
                       for dt in range(NDT)]
                outp = [fpp.tile([128, LQ], f32, name=f"outp{dt}",
                                 tag=f"outp{dt}") for dt in range(NDT)]
                ln_tail(psx, pss, [x1[dt][:] for dt in range(NDT)], LQ,
                        None, None, [xTb[dt][:] for dt in range(NDT)],
                        rstd_bias=lnxs[:])
                x8 = [hp.tile([128, 2, LQ], f8, name=f"x8{dp}", tag=f"x8{dp}",
                              bufs=1) for dp in range(2)]
                for dt in range(NDT):
                    nc.scalar.activation(x8[dt // 2][:, dt % 2, :],
                                         xTb[dt][:], AF.Identity)
                pf2 = [ps.tile([128, LQ], f32, name=f"pf2_{i}", tag="ps")
                       for i in range(NDT)]
                h8t = None
                for ft in range(NFT):
                    ph1 = ps.tile([128, LQ], f32, name="ph1", tag="ps")
                    for dp in range(2):
                        nc.tensor.matmul(
                            ph1[:], ff1_sb[dp][:, :, ft * 128:(ft + 1) * 128],
                            x8[dp][:, :, :],
                            start=(dp == 0), stop=(dp == 1), perf_mode=DR)
                    if ft % 2 == 0:
                        h8t = hp.tile([128, 2, LQ], f8, name="h8", tag="h8")
                    nc.scalar.activation(h8t[:, ft % 2, :], ph1[:], AF.Relu,
                                         bias=ff1b_sb[:, ft:ft + 1],
                                         scale=1.0 / (WSCALE * XSCALE))
                    if ft % 2 == 1:
                        fp_ = ft // 2
                        for dot in range(NDT):
                            nc.tensor.matmul(
                                pf2[dot][:],
                                ff2_sb[fp_][:, :, dot * 128:(dot + 1) * 128],
                                h8t[:, :, :], start=(fp_ == 0),
                                stop=(fp_ == NFT // 2 - 1), perf_mode=DR)

                # ff2 bias + residual, LN2 stats + apply + store, in two
                # l-halves so half 0's serial LN2 chain overlaps half 1's.
                for lh in range(2):
                    lo = lh * LH
                    x2 = []
                    psx2 = ps_st.tile([1, LH], f32, name="pstx2", tag="stx")
                    pss2 = ps_st.tile([1, LH], f32, name="psts2", tag="sts")
                    for dot in range(NDT):
                        t = fp.tile([128, LH], f32r, name=f"x2{dot}",
                                    tag=f"x2{dot}")
                        nc.vector.scalar_tensor_tensor(
                            t[:], pf2[dot][:, lo:lo + LH],
                            ff2b_sb[:, dot:dot + 1],
                            xTb[dot][:, lo:lo + LH], ALU.add, ALU.add)
                        x2.append(t)
                        nc.tensor.matmul(psx2[:], ones_col[:], t[:],
                                         start=(dot == 0), stop=(dot == NDT - 1))
                        sqt = ap_sq.tile([128, LH], f32r, name="sqB", tag="sqB")
                        nc.scalar.activation(sqt[:], t[:].bitcast(f32),
                                             AF.Square)
                        nc.tensor.matmul(pss2[:], ones_col[:], sqt[:],
                                         start=(dot == 0), stop=(dot == NDT - 1))

                    ln_tail(psx2[:], pss2[:], [t[:] for t in x2], LH,
                            ln2g_sb, ln2b_sb,
                            [o[:, lo:lo + LH] for o in outp])
                    for dt in range(NDT):
                        nc.sync.dma_start(
                            outT_d[dt * 128:(dt + 1) * 128, lo:lo + LH],
                            outp[dt][:, lo:lo + LH])

    nc.compile()
    _PROG_CACHE[key] = nc
    return nc


def _col_tiles(vec):
    """(N,) -> (128, N//128) with [p, j] = vec[j*128 + p]."""
    return np.ascontiguousarray(vec.reshape(-1, 128).T.astype(np.float32))


def _prep_inputs(src, mask, attn_mask, in_proj_w, in_proj_b, ln1_g, ln1_b,
                 ff1_w, ff1_b, ff2_w, ff2_b, ln2_g, ln2_b):
    src = np.asarray(src, np.float32)
    mask = np.asarray(mask, bool)
    attn_mask = np.asarray(attn_mask, bool)
    w = np.asarray(in_proj_w, np.float32)
    b = np.asarray(in_proj_b, np.float32)
    # A_h = Wq_h^T @ Wk_h  (D x D per head), c_h = Wk_h^T @ bq_h
    aT = np.empty((D, H * D), np.float32)
    cT = np.empty((D, H), np.float32)
    for h in range(H):
        wq_h = w[h * D:(h + 1) * D]                  # (D, D)
        wk_h = w[H * D + h * D:H * D + (h + 1) * D]  # (D, D)
        bq_h = b[h * D:(h + 1) * D]
        aT[:, h * D:(h + 1) * D] = (wq_h.T @ wk_h) * ASCALE
        cT[:, h] = (wk_h.T @ bq_h) * ASCALE
    cT = cT.reshape(NDT, 128, H).transpose(1, 0, 2).reshape(128, NDT * H)
    wvT = np.asarray(in_proj_w[2 * H * D:], np.float32).T
    bvbc = np.ascontiguousarray(
        np.broadcast_to(b[2 * H * D:].astype(np.float32), (128, H * DV)))
    shared = {
        "aT": aT.astype(F8),
        "cT": np.ascontiguousarray(cT).astype(BF),
        "wvT": np.ascontiguousarray(wvT).astype(BF),
        "bvbc": bvbc,
        "ff1T": np.ascontiguousarray(
            (np.asarray(ff1_w, np.float64) * np.asarray(ln1_g, np.float64)[None, :]
             * WSCALE).T.astype(np.float32)).astype(F8),
        "ff2T": np.ascontiguousarray(
            (np.asarray(ff2_w, np.float32) * XSCALE).T).astype(F8),
        "ff1b": _col_tiles(
            (np.asarray(ff1_w, np.float64) @ np.asarray(ln1_b, np.float64)
             + np.asarray(ff1_b, np.float64)).astype(np.float32)),
        "ff2b": _col_tiles(np.asarray(ff2_b, np.float32) * XSCALE),
        "ln2g": _col_tiles(np.asarray(ln2_g, np.float32)),
        "ln2b": _col_tiles(np.asarray(ln2_b, np.float32)),
    }
    in_maps = []
    for c in range(NCORES):
        bidx, half = divmod(c, 2)
        perm = np.r_[half * LQ:L, 0:half * LQ]
        srcT = np.ascontiguousarray(src[bidx].T[:, perm])
        # fp8 copy packed for DoubleRow: [ki, (dp*2+ko)*L + l]
        src8 = np.ascontiguousarray(
            srcT.reshape(2, 2, 128, L).transpose(2, 0, 1, 3).reshape(128, 4 * L)
        ).astype(F8)
        # combined additive mask bias, transposed to [m_rot, l_own]
        cm = mask[bidx][None, :] | attn_mask[half * LQ:(half + 1) * LQ, :]
        biasT = np.ascontiguousarray(
            (cm[:, perm].T.astype(np.float32) * (NINF * ASCALE))).astype(BF)
        m = dict(shared)
        m["srcT"] = srcT
        m["src8"] = src8
        m["srcb"] = srcT.astype(BF)
        m["biasT"] = biasT
        in_maps.append(m)
    return in_maps


def _run(inputs, trace=False):
    nc = _build_program()
    in_maps = _prep_inputs(**inputs)
    for attempt in range(3):
        try:
            res = run_bass_kernel_spmd(nc, in_maps, list(range(NCORES)),
                                       trace=trace)
            break
        except Exception:  # transient NRT device errors observed
            if attempt == 2:
                raise
    out = np.empty((B, L, D), np.float32)
    for c in range(NCORES):
        bidx, half = divmod(c, 2)
        out[bidx, half * LQ:(half + 1) * LQ, :] = res.results[c]["outT"].T
    return out, res


def kernel(**inputs):
    out, _ = _run(inputs, trace=False)
    return out


if __name__ == "__main__":
    import reference
    inputs = {k: np.asarray(v) for k, v in reference.setup_inputs().items()}
    out = kernel(**inputs)
    print("out", out.shape, out.dtype)


# revision 81
# speedup vs baseline: 1.2271x; 1.2271x over previous
"""Trainium2 Bass kernel for an AttNHP transformer layer.

Shapes (hardcoded): src (4, 1024, 512), nhead=8 with full-width (512) q/k per
head, dim_value 64, ffn 2048.  Runs SPMD on 8 NeuronCores: core c handles
batch c//2, query-token half c%2.  The host rotates the token axis per core so
one shared program works for every core, computes the combined additive
attention-mask bias, and pre-transposes all weights.

Attention trick: scores[l,m] = q_l.k_m with q=Wq s+bq, k=Wk s+bk equals
s_l^T A s_m + g(m) + f(l) + const with A = Wq^T Wk and g(m) = (Wk^T bq).s_m.
The f(l)+const part is constant along the softmax axis and cancels exactly,
so the device only computes u = A^T s (own tokens) and scores = u.s — the
whole K projection disappears.  A and c = Wk^T bq are precomputed on host.
All heavy matmuls run in bf16 (fast-weight-load eligible); residual/LN math
stays fp32.
"""

import os
import sys
import numpy as np
from contextlib import ExitStack

for _p in ("/opt/trn_rl_repo", "/root/.axon_site/_ro/trn_rl_repo"):
    if os.path.isdir(_p) and _p not in sys.path:
        sys.path.append(_p)

import concourse.bass as bass  # noqa: E402
import concourse.tile as tile  # noqa: E402
from concourse import bacc, mybir  # noqa: E402
from concourse.bass_utils import run_bass_kernel_spmd  # noqa: E402

import ml_dtypes  # noqa: E402

f32 = mybir.dt.float32
f32r = mybir.dt.float32r
bf16 = mybir.dt.bfloat16
f8 = mybir.dt.float8e4
AF = mybir.ActivationFunctionType
ALU = mybir.AluOpType
DR = mybir.MatmulPerfMode.DoubleRow

B, L, D, H, DV, F = 4, 1024, 512, 8, 64, 2048
LQ = L // 2          # tokens per core
NCORES = 8
NDT = D // 128       # 4  d-tiles
NMT = L // 128       # 8  key-token tiles
NFT = F // 128       # 16 ffn tiles
SCALE = 1.0 / float(np.sqrt(np.float32(D)))
NINF = -1000000.0
BF = ml_dtypes.bfloat16
F8 = ml_dtypes.float8_e4m3
ASCALE = 64.0   # A,c,biasT scaled by this; compensated in the exp scale
WSCALE = 64.0   # ff1 weight scale into fp8 normal range
XSCALE = 16.0   # LN1 output fold (x2/LN2 are scale-invariant)

_PROG_CACHE = {}
_ACT_PATCHED = [False]


def _patch_act_tables():
    """Force every ACTIVATE onto the natural_log_exp_and_others table set.

    The default chooser maps each function to its first-containing set
    (Exp -> exp_and_others, Ln -> natural_log), which reloads tables twice
    per attention head.  Emptying every other set (order, hence set ids,
    preserved) makes the one set that contains Exp+Ln+Relu+Identity+Square
    the only candidate: one ACT_TABLE_LOAD for the whole kernel.
    """
    if _ACT_PATCHED[0]:
        return
    orig = bacc.get_activation_tables

    def patched(arch):
        t = dict(orig(arch))
        keep = t.get("natural_log_exp_and_others")
        if not keep:
            return t
        for k in t:
            if k != "natural_log_exp_and_others":
                t[k] = t[k] - keep
        return t

    bacc.get_activation_tables = patched
    _ACT_PATCHED[0] = True


def _build_program():
    key = "prog"
    if key in _PROG_CACHE:
        return _PROG_CACHE[key]
    _patch_act_tables()

    nc = bacc.Bacc("TRN2", target_bir_lowering=False, debug=False,
                   num_devices=NCORES)

    def din(name, shape, dt=f32):
        return nc.dram_tensor(name, list(shape), dt, kind="ExternalInput").ap()

    srcT_d = din("srcT", (D, L))
    biasT_d = din("biasT", (L, LQ), bf16)
    aT_d = din("aT", (D, H * D), f8)         # ASCALE * Wq_h^T Wk_h
    cT_d = din("cT", (128, NDT * H), bf16)   # ASCALE * Wk_h^T bq_h, tiled
    wvT_d = din("wvT", (D, H * DV), bf16)
    bvbc_d = din("bvbc", (128, H * DV))
    ff1T_d = din("ff1T", (D, F), f8)         # WSCALE * ff1_w*ln1_g, transposed
    ff2T_d = din("ff2T", (F, D), f8)         # XSCALE * ff2_w, transposed
    ff1b_d = din("ff1b", (128, NFT))
    ff2b_d = din("ff2b", (128, NDT))
    ln2g_d = din("ln2g", (128, NDT))
    ln2b_d = din("ln2b", (128, NDT))
    outT_d = nc.dram_tensor("outT", [D, LQ], f32, kind="ExternalOutput").ap()

    with tile.TileContext(nc) as tc, ExitStack() as ctx:
        pp = ctx.enter_context(tc.tile_pool(name="pp", bufs=1))
        ps = ctx.enter_context(tc.tile_pool(name="ps", bufs=6, space="PSUM"))
        ps_st = ctx.enter_context(tc.tile_pool(name="psst", bufs=1,
                                               space="PSUM"))
        ap_x1 = ctx.enter_context(tc.tile_pool(name="x1p", bufs=1))
        ap_sq = ctx.enter_context(tc.tile_pool(name="sqp", bufs=2))

        def load_const(name, dram, shape, dt=f32):
            t = pp.tile(list(shape), dt, name=name, tag=name)
            nc.sync.dma_start(t[:], dram[:])
            return t

        onesf = pp.tile([128, 8], bf16, name="onesf", tag="onesf")
        nc.vector.memset(onesf[:], 1.0)
        onesc = pp.tile([128, 1], f32, name="onesc", tag="onesc")
        nc.vector.memset(onesc[:], 1.0)
        ones_col = pp.tile([128, 1], f32r, name="ones", tag="ones")
        nc.vector.tensor_copy(ones_col[:], onesc[:])
        onesr = pp.tile([1, 128], f32, name="onesr", tag="onesr")
        nc.vector.memset(onesr[:], 1.0)
        ones_row = pp.tile([1, 128], f32r, name="onesrr", tag="onesrr")
        nc.vector.tensor_copy(ones_row[:], onesr[:])
        eps30 = pp.tile([1, 1], f32, name="eps30", tag="eps30")
        nc.vector.memset(eps30[:], 1e-30)
        epsln = pp.tile([1, 1], f32, name="epsln", tag="epsln")
        nc.vector.memset(epsln[:], 1e-5)
        lnxs = pp.tile([1, 1], f32, name="lnxs", tag="lnxs")
        nc.vector.memset(lnxs[:], float(np.log(XSCALE)))
        lnws = pp.tile([1, 1], f32, name="lnws", tag="lnws")
        nc.vector.memset(lnws[:], float(np.log(WSCALE * XSCALE)))

        # fp32 src (residual path) + bf16 cast (vproj/g) + fp8 DoubleRow-
        # packed cast (uproj moving + scores stationary): src8[dp][ki,ko,l]
        # = src[dp*256 + ko*128 + ki, l].
        srcT = []
        srcb = []
        for dt in range(NDT):
            t = pp.tile([128, L], f32, name=f"srcT{dt}", tag=f"srcT{dt}")
            nc.sync.dma_start(t[:, 0:LQ], srcT_d[dt * 128:(dt + 1) * 128, 0:LQ])
            srcT.append(t)
            b = pp.tile([128, L], bf16, name=f"srcb{dt}", tag=f"srcb{dt}")
            srcb.append(b)
        src8 = [pp.tile([128, 2, L], f8, name=f"src8{dp}", tag=f"src8{dp}")
                for dp in range(2)]
        for dt in range(NDT):
            nc.vector.tensor_copy(src8[dt // 2][:, dt % 2, 0:LQ],
                                  srcT[dt][:, 0:LQ])
        for dt in range(NDT):
            nc.vector.tensor_copy(srcb[dt][:, 0:LQ], srcT[dt][:, 0:LQ])

        vaug = [pp.tile([128, H * 65], bf16, name=f"vaug{mt}", tag=f"vaug{mt}")
                for mt in range(NMT)]
        saT = [pp.tile([128, LQ], f32, name=f"saT{dt}", tag=f"saT{dt}")
               for dt in range(NDT)]
        biasT = [pp.tile([128, LQ], bf16, name=f"biasT{mt}", tag=f"biasT{mt}")
                 for mt in range(NMT)]
        g_sb = pp.tile([128, NMT * H], f32, name="g_sb", tag="g_sb")

        with ExitStack() as actx:
            ap_wa = actx.enter_context(tc.tile_pool(name="wap", bufs=8))
            ap_u = actx.enter_context(tc.tile_pool(name="up", bufs=8))
            ap_ex = actx.enter_context(tc.tile_pool(name="exq", bufs=8))
            ap_sc = actx.enter_context(tc.tile_pool(name="scq", bufs=4))
            ap_row = actx.enter_context(tc.tile_pool(name="rowp", bufs=2))

            def emit_uproj(h, mid_emit=None):
                a_t = []
                for dp in range(2):
                    t = ap_wa.tile([128, 2, D], f8, name="wa", tag="wa")
                    for ko in range(2):
                        r0 = dp * 256 + ko * 128
                        nc.sync.dma_start(t[:, ko, :],
                                          aT_d[r0:r0 + 128, h * D:(h + 1) * D])
                    a_t.append(t)
                u = [ap_u.tile([128, 2, LQ], f8, name="uh", tag="uh")
                     for _ in range(2)]
                for pt in range(NDT):
                    pq = ps.tile([128, LQ], f32, name="psw", tag="ps")
                    for dp in range(2):
                        nc.tensor.matmul(pq[:],
                                         a_t[dp][:, :, pt * 128:(pt + 1) * 128],
                                         src8[dp][:, :, 0:LQ],
                                         start=(dp == 0), stop=(dp == 1),
                                         perf_mode=DR)
                    if pt % 2 == 0:
                        nc.scalar.activation(u[pt // 2][:, pt % 2, :], pq[:],
                                             AF.Identity)
                    else:
                        nc.vector.tensor_copy(u[pt // 2][:, pt % 2, :], pq[:])
                    if mid_emit is not None and pt == 0:
                        mid_emit()
                return u

            def emit_scores(h, u):
                ex = []
                for mt in range(NMT):
                    psc = ps.tile([128, LQ], f32, name="psw", tag="ps")
                    for dp in range(2):
                        nc.tensor.matmul(psc[:],
                                         src8[dp][:, :, mt * 128:(mt + 1) * 128],
                                         u[dp][:, :, :],
                                         start=(dp == 0), stop=(dp == 1),
                                         perf_mode=DR)
                    sct = ap_sc.tile([128, LQ], bf16, name="sc", tag="sc")
                    nc.vector.scalar_tensor_tensor(
                        sct[:], psc[:], g_sb[:, mt * H + h:mt * H + h + 1],
                        biasT[mt][:], ALU.add, ALU.add)
                    et = ap_ex.tile([128, LQ], bf16, name="ex", tag="ex")
                    nc.scalar.activation(et[:], sct[:], AF.Exp,
                                         scale=SCALE / ASCALE)
                    ex.append(et)
                return ex

            def dummy_mm(moving, f32_=False):
                """Tiny dead matmul: keeps the PE HAM activity window busy
                through serial DVE/ACT chains so the clock is not
                re-throttled to 1.2 GHz right before the next matmul burst.
                `moving` must be a [1, >=64] (row) or [128, >=64] AP."""
                p = moving.partition_size()
                dmt = ps.tile([1, 64], f32, name="dum", tag="ps")
                if f32_:
                    stat = eps30[:] if p == 1 else onesc[0:p, :]
                else:
                    stat = ones_row[0:1, 0:1] if p == 1 else ones_col[0:p, :]
                nc.tensor.matmul(dmt[:], stat, moving[:, 0:64],
                                 start=True, stop=True)

            def emit_pv_mm(h, ex, warm=False):
                ppv = ps.tile([65, LQ], f32, name="ppv", tag="ps")
                for mt in range(NMT):
                    nc.tensor.matmul(ppv[:], vaug[mt][:, h * 65:(h + 1) * 65],
                                     ex[mt][:], start=(mt == 0), stop=(mt == NMT - 1))
                lt = ap_row.tile([1, LQ], f32, name="lt", tag="lt")
                nc.scalar.activation(lt[:], ppv[64:65, :], AF.Ln, bias=eps30[:])
                if warm:
                    dummy_mm(lt[0:1, :], f32_=True)
                rt = ap_row.tile([1, LQ], f32r, name="rt", tag="rt")
                nc.scalar.activation(rt[:], lt[:], AF.Exp, scale=-1.0)
                return ppv, rt

            def emit_norm(h, ppv, rt, warm=False):
                prb = ps.tile([64, LQ], f32, name="prb", tag="ps")
                nc.tensor.matmul(prb[:], ones_row[0:1, 0:64], rt[:],
                                 start=True, stop=True)
                rbc = ap_row.tile([64, LQ], f32, name="rbc", tag="rbc")
                nc.vector.tensor_copy(rbc[:], prb[:])
                if warm:
                    dummy_mm(rbc[:], f32_=True)
                sat = saT[h // 2]
                r0 = (h % 2) * 64
                nc.vector.tensor_tensor(sat[r0:r0 + 64, :], ppv[0:64, :], rbc[:],
                                        ALU.mult)

            # ---- startup-ordered emission ----
            def _late_src():
                for dt in range(NDT):
                    nc.sync.dma_start(srcT[dt][:, LQ:L],
                                      srcT_d[dt * 128:(dt + 1) * 128, LQ:L])
                for dt in range(NDT):
                    nc.vector.tensor_copy(src8[dt // 2][:, dt % 2, LQ:L],
                                          srcT[dt][:, LQ:L])
                for dt in range(NDT):
                    nc.vector.tensor_copy(srcb[dt][:, LQ:L], srcT[dt][:, LQ:L])

            u0 = emit_uproj(0, mid_emit=_late_src)

            # g(m): per-head additive score bias from Wk^T bq
            cT_sb = load_const("cT", cT_d, (128, NDT * H), bf16)
            g_ps = ps.tile([128, NMT * H], f32, name="gps", tag="ps")
            for mt in range(NMT):
                for dt in range(NDT):
                    nc.tensor.matmul(g_ps[:, mt * H:(mt + 1) * H],
                                     srcb[dt][:, mt * 128:(mt + 1) * 128],
                                     cT_sb[:, dt * H:(dt + 1) * H],
                                     start=(dt == 0), stop=(dt == NDT - 1))
            nc.vector.tensor_copy(g_sb[:], g_ps[:])

            # V projection (natural layout [m, j]) + ones column
            wv = []
            for dt in range(NDT):
                t = pp.tile([128, H * DV], bf16, name=f"wv{dt}", tag=f"wv{dt}")
                nc.sync.dma_start(t[:], wvT_d[dt * 128:(dt + 1) * 128, :])
                wv.append(t)
            bvbc_sb = load_const("bvbc", bvbc_d, (128, H * DV))
            for mt in range(NMT):
                pv = ps.tile([128, H * DV], f32, name="psv", tag="ps")
                for dt in range(NDT):
                    nc.tensor.matmul(pv[:], srcb[dt][:, mt * 128:(mt + 1) * 128],
                                     wv[dt][:], start=(dt == 0), stop=(dt == NDT - 1))
                va_v = vaug[mt][:].rearrange("p (h c) -> p h c", c=65)[:, :, 0:64]
                pv_v = pv[:].rearrange("p (h c) -> p h c", c=64)
                bv_v = bvbc_sb[:].rearrange("p (h c) -> p h c", c=64)
                nc.vector.tensor_tensor(va_v, pv_v, bv_v, ALU.add)
                va_ones = vaug[mt][:].rearrange("p (h c) -> p h c", c=65)[:, :, 64:65]
                nc.vector.tensor_copy(va_ones,
                                      onesf[:].rearrange("p (h o) -> p h o", o=1))

            # remaining big/late loads
            for mt in range(NMT):
                nc.sync.dma_start(biasT[mt][:],
                                  biasT_d[mt * 128:(mt + 1) * 128, :])
            ff1b_sb = load_const("ff1b", ff1b_d, (128, NFT))
            ff2b_sb = load_const("ff2b", ff2b_d, (128, NDT))
            ln2g_sb = load_const("ln2g", ln2g_d, (128, NDT))
            ln2b_sb = load_const("ln2b", ln2b_d, (128, NDT))
            # all FFN weights resident in SBUF (fp8, DoubleRow-packed); DMA
            # issue is chunked and interleaved into the head loop so per-head
            # aT prefetches are not stuck behind the weight traffic.
            ff1_sb = [pp.tile([128, 2, F], f8, name=f"ff1w{dp}", tag=f"ff1w{dp}")
                      for dp in range(2)]
            ff2_sb = [pp.tile([128, 2, D], f8, name=f"ff2w{fp_}", tag=f"ff2w{fp_}")
                      for fp_ in range(NFT // 2)]
            ff_chunks = []
            for dp in range(2):
                for ko in range(2):
                    r0 = dp * 256 + ko * 128
                    for hf in range(2):
                        ff_chunks.append(
                            (ff1_sb[dp][:, ko, hf * 1024:(hf + 1) * 1024],
                             ff1T_d[r0:r0 + 128, hf * 1024:(hf + 1) * 1024]))
            for fp_ in range(NFT // 2):
                for ko in range(2):
                    r0 = fp_ * 256 + ko * 128
                    ff_chunks.append((ff2_sb[fp_][:, ko, :],
                                      ff2T_d[r0:r0 + 128, :]))

            def issue_ff_chunks(n):
                while n > 0 and ff_chunks:
                    dst, src_ = ff_chunks.pop(0)
                    nc.sync.dma_start(dst, src_)
                    n -= 1

            # heads pipelined: scores(h) -> norm(h-1) -> uproj(h+1) -> pv(h)
            # LN1 stats matmuls stream into the head loop as x1 tiles finish.
            psx = ps_st.tile([1, LQ], f32, name="st1x", tag="stx")
            pss = ps_st.tile([1, LQ], f32, name="st1s", tag="sts")
            x1 = [None] * NDT
            sqs = [None] * NDT
            u = u0
            pend = None

            def flush_norm(warm=False):
                hp_, ppv_, rt_ = pend
                emit_norm(hp_, ppv_, rt_, warm=warm)
                if hp_ % 2 == 1:
                    dt = hp_ // 2
                    t = ap_x1.tile([128, LQ], f32r, name=f"x1{dt}", tag=f"x1{dt}")
                    nc.vector.tensor_tensor(t[:], srcT[dt][:, 0:LQ],
                                            saT[dt][:], ALU.add)
                    x1[dt] = t
                    if warm:
                        dummy_mm(t[:])
                    sqt = ap_sq.tile([128, LQ], f32r, name=f"sq{dt}",
                                     tag=f"sq{dt}", bufs=1)
                    nc.scalar.activation(sqt[:], t[:].bitcast(f32), AF.Square)
                    sqs[dt] = sqt
                    nc.tensor.matmul(psx, ones_col[:], t[:],
                                     start=(dt == 0), stop=(dt == NDT - 1))
                    nc.tensor.matmul(pss, ones_col[:], sqt[:],
                                     start=(dt == 0), stop=(dt == NDT - 1))

            for h in range(H):
                ex = emit_scores(h, u)
                if pend is not None:
                    flush_norm()
                u = emit_uproj(h + 1) if h + 1 < H else None
                issue_ff_chunks(4)
                ppv, rt = emit_pv_mm(h, ex, warm=(h == H - 1))
                pend = (h, ppv, rt)
            flush_norm(warm=True)
            issue_ff_chunks(len(ff_chunks))

        # LN1 tail (stats -> mean/rstd -> apply), then FFN+LN2 in two
        # 256-token halves so half 0's serial tail overlaps half 1's matmuls.
        if True:
            with ExitStack() as fctx:
                fpp = fctx.enter_context(tc.tile_pool(name="fpp", bufs=1))
                fp = fctx.enter_context(tc.tile_pool(name="fp", bufs=2))
                hp = fctx.enter_context(tc.tile_pool(name="hp", bufs=4))
                LH = LQ // 2

                def ln_tail(psx_, pss_, xs, ln_, g_sb_, b_sb_, out_aps,
                            rstd_bias=None, spec=None):
                    comb = fp.tile([1, 2 * ln_], f32r, name="comb", tag="comb")
                    mean = comb[0:1, 0:ln_]
                    nc.vector.tensor_scalar(mean, psx_, 1.0 / D, None,
                                            ALU.mult)
                    pmb = ps.tile([128, ln_], f32, name="pmb", tag="ps")
                    nc.tensor.matmul(pmb[:], ones_row[:], comb[0:1, 0:ln_],
                                     start=True, stop=True)
                    m2 = fp.tile([1, ln_], f32, name="m2", tag="m2")
                    nc.vector.tensor_tensor(m2[:], mean.bitcast(f32),
                                            mean.bitcast(f32), ALU.mult)
                    var = fp.tile([1, ln_], f32, name="var", tag="var")
                    nc.vector.scalar_tensor_tensor(var[:], pss_, 1.0 / D,
                                                   m2[:], ALU.mult, ALU.subtract)
                    lnv = fp.tile([1, ln_], f32, name="lnv", tag="lnv")
                    nc.scalar.activation(lnv[:], var[:], AF.Ln, bias=epsln[:])
                    if spec is not None:
                        # R rows for the speculative-ff1 rank-1 fixup:
                        # row0 = -XSCALE*mean, row1 = (WSCALE*XSCALE)/r.
                        # Row 1 must land on partition 1 — engines can only
                        # write partition-0-based APs, so hop via DMA.
                        nc.vector.tensor_scalar(spec[0:1, :],
                                                mean.bitcast(f32), -XSCALE,
                                                None, ALU.mult)
                        r1t = fp.tile([1, ln_], bf16, name="r1t", tag="r1t")
                        nc.scalar.activation(r1t[:], lnv[:], AF.Exp,
                                             scale=0.5, bias=lnws[:])
                        nc.sync.dma_start(spec[1:2, :], r1t[:])
                    nc.scalar.activation(comb[0:1, ln_:2 * ln_], lnv[:], AF.Exp,
                                         scale=-0.5,
                                         bias=0.0 if rstd_bias is None
                                         else rstd_bias)
                    prs = ps.tile([128, ln_], f32, name="prs", tag="ps")
                    nc.tensor.matmul(prs[:], ones_row[:], comb[0:1, ln_:2 * ln_],
                                     start=True, stop=True)
                    for dt in range(NDT):
                        t1 = fp.tile([128, ln_], f32, name="lnt1", tag="lnt1")
                        nc.vector.tensor_tensor(t1[:], xs[dt].bitcast(f32),
                                                pmb[:], ALU.subtract)
                        if g_sb_ is None:
                            nc.vector.tensor_tensor(out_aps[dt], t1[:], prs[:],
                                                    ALU.mult)
                        else:
                            t2 = fp.tile([128, ln_], f32, name="lnt2", tag="lnt2")
                            nc.vector.tensor_tensor(t2[:], t1[:], prs[:], ALU.mult)
                            nc.scalar.activation(out_aps[dt], t2[:], AF.Identity,
                                                 bias=b_sb_[:, dt:dt + 1],
                                                 scale=g_sb_[:, dt:dt + 1])
                    return prs

                xTb = [fpp.tile([128, LQ], bf16, name=f"xTb{dt}", tag=f"xTb{dt}")
                       for dt in range(NDT)]
                outp = [fpp.tile([128, LQ], f32, name=f"outp{dt}",
                                 tag=f"outp{dt}") for dt in range(NDT)]
                ln_tail(psx, pss, [x1[dt][:] for dt in range(NDT)], LQ,
                        None, None, [xTb[dt][:] for dt in range(NDT)],
                        rstd_bias=lnxs[:])
                x8 = [hp.tile([128, 2, LQ], f8, name=f"x8{dp}", tag=f"x8{dp}",
                              bufs=1) for dp in range(2)]
                for dt in range(NDT):
                    nc.scalar.activation(x8[dt // 2][:, dt % 2, :],
                                         xTb[dt][:], AF.Identity)
                pf2 = [ps.tile([128, LQ], f32, name=f"pf2_{i}", tag="ps")
                       for i in range(NDT)]
                h8t = None
                for ft in range(NFT):
                    ph1 = ps.tile([128, LQ], f32, name="ph1", tag="ps")
                    for dp in range(2):
                        nc.tensor.matmul(
                            ph1[:], ff1_sb[dp][:, :, ft * 128:(ft + 1) * 128],
                            x8[dp][:, :, :],
                            start=(dp == 0), stop=(dp == 1), perf_mode=DR)
                    if ft % 2 == 0:
                        h8t = hp.tile([128, 2, LQ], f8, name="h8", tag="h8")
                    nc.scalar.activation(h8t[:, ft % 2, :], ph1[:], AF.Relu,
                                         bias=ff1b_sb[:, ft:ft + 1],
                                         scale=1.0 / (WSCALE * XSCALE))
                    if ft % 2 == 1:
                        fp_ = ft // 2
                        for dot in range(NDT):
                            nc.tensor.matmul(
                                pf2[dot][:],
                                ff2_sb[fp_][:, :, dot * 128:(dot + 1) * 128],
                                h8t[:, :, :], start=(fp_ == 0),
                                stop=(fp_ == NFT // 2 - 1), perf_mode=DR)

                # ff2 bias + residual, LN2 stats + apply + store, in two
                # l-halves so half 0's serial LN2 chain overlaps half 1's.
                for lh in range(2):
                    lo = lh * LH
                    x2 = []
                    psx2 = ps_st.tile([1, LH], f32, name="pstx2", tag="stx")
                    pss2 = ps_st.tile([1, LH], f32, name="psts2", tag="sts")
                    for dot in range(NDT):
                        t = fp.tile([128, LH], f32r, name=f"x2{dot}",
                                    tag=f"x2{dot}")
                        nc.vector.scalar_tensor_tensor(
                            t[:], pf2[dot][:, lo:lo + LH],
                            ff2b_sb[:, dot:dot + 1],
                            xTb[dot][:, lo:lo + LH], ALU.add, ALU.add)
                        x2.append(t)
                        nc.tensor.matmul(psx2[:], ones_col[:], t[:],
                                         start=(dot == 0), stop=(dot == NDT - 1))
                        sqt = ap_sq.tile([128, LH], f32r, name="sqB", tag="sqB")
                        nc.scalar.activation(sqt[:], t[:].bitcast(f32),
                                             AF.Square)
                        nc.tensor.matmul(pss2[:], ones_col[:], sqt[:],
                                         start=(dot == 0), stop=(dot == NDT - 1))

                    ln_tail(psx2[:], pss2[:], [t[:] for t in x2], LH,
                            ln2g_sb, ln2b_sb,
                            [o[:, lo:lo + LH] for o in outp])
                    for dt in range(NDT):
                        nc.sync.dma_start(
                            outT_d[dt * 128:(dt + 1) * 128, lo:lo + LH],
                            outp[dt][:, lo:lo + LH])

    nc.compile()
    _PROG_CACHE[key] = nc
    return nc


def _col_tiles(vec):
    """(N,) -> (128, N//128) with [p, j] = vec[j*128 + p]."""
    return np.ascontiguousarray(vec.reshape(-1, 128).T.astype(np.float32))


def _prep_inputs(src, mask, attn_mask, in_proj_w, in_proj_b, ln1_g, ln1_b,
                 ff1_w, ff1_b, ff2_w, ff2_b, ln2_g, ln2_b):
    src = np.asarray(src, np.float32)
    mask = np.asarray(mask, bool)
    attn_mask = np.asarray(attn_mask, bool)
    w = np.asarray(in_proj_w, np.float32)
    b = np.asarray(in_proj_b, np.float32)
    # A_h = Wq_h^T @ Wk_h  (D x D per head), c_h = Wk_h^T @ bq_h
    aT = np.empty((D, H * D), np.float32)
    cT = np.empty((D, H), np.float32)
    for h in range(H):
        wq_h = w[h * D:(h + 1) * D]                  # (D, D)
        wk_h = w[H * D + h * D:H * D + (h + 1) * D]  # (D, D)
        bq_h = b[h * D:(h + 1) * D]
        aT[:, h * D:(h + 1) * D] = (wq_h.T @ wk_h) * ASCALE
        cT[:, h] = (wk_h.T @ bq_h) * ASCALE
    cT = cT.reshape(NDT, 128, H).transpose(1, 0, 2).reshape(128, NDT * H)
    wvT = np.asarray(in_proj_w[2 * H * D:], np.float32).T
    bvbc = np.ascontiguousarray(
        np.broadcast_to(b[2 * H * D:].astype(np.float32), (128, H * DV)))
    shared = {
        "aT": aT.astype(F8),
        "cT": np.ascontiguousarray(cT).astype(BF),
        "wvT": np.ascontiguousarray(wvT).astype(BF),
        "bvbc": bvbc,
        "ff1T": np.ascontiguousarray(
            (np.asarray(ff1_w, np.float64) * np.asarray(ln1_g, np.float64)[None, :]
             * WSCALE).T.astype(np.float32)).astype(F8),
        "ff2T": np.ascontiguousarray(
            (np.asarray(ff2_w, np.float32) * XSCALE).T).astype(F8),
        "ff1b": _col_tiles(
            (np.asarray(ff1_w, np.float64) @ np.asarray(ln1_b, np.float64)
             + np.asarray(ff1_b, np.float64)).astype(np.float32)),
        "ff2b": _col_tiles(np.asarray(ff2_b, np.float32) * XSCALE),
        "ln2g": _col_tiles(np.asarray(ln2_g, np.float32)),
        "ln2b": _col_tiles(np.asarray(ln2_b, np.float32)),
    }
    in_maps = []
    for c in range(NCORES):
        bidx, half = divmod(c, 2)
        perm = np.r_[half * LQ:L, 0:half * LQ]
        srcT = np.ascontiguousarray(src[bidx].T[:, perm])
        # combined additive mask bias, transposed to [m_rot, l_own]
        cm = mask[bidx][None, :] | attn_mask[half * LQ:(half + 1) * LQ, :]
        biasT = np.ascontiguousarray(
            (cm[:, perm].T.astype(np.float32) * (NINF * ASCALE))).astype(BF)
        m = dict(shared)
        m["srcT"] = srcT
        m["biasT"] = biasT
        in_maps.append(m)
    return in_maps


def _run(inputs, trace=False):
    nc = _build_program()
    in_maps = _prep_inputs(**inputs)
    for attempt in range(3):
        try:
            res = run_bass_kernel_spmd(nc, in_maps, list(range(NCORES)),
                                       trace=trace)
            break
        except Exception:  # transient NRT device errors observed
            if attempt == 2:
                raise
    out = np.empty((B, L, D), np.float32)
    for c in range(NCORES):
        bidx, half = divmod(c, 2)
        out[bidx, half * LQ:(half + 1) * LQ, :] = res.results[c]["outT"].T
    return out, res


def kernel(**inputs):
    out, _ = _run(inputs, trace=False)
    return out


if __name__ == "__main__":
    import reference
    inputs = {k: np.asarray(v) for k, v in reference.setup_inputs().items()}
    out = kernel(**inputs)
    print("out", out.shape, out.dtype)
